# revision 3
# baseline (speedup 1.0000x reference)
"""Trainium2 Bass kernel for nn_Aggregation (sparse block-diagonal attention).

Computation (see reference): a single learned query vector attends, per
(sentence, batch), over that sentence's 32 entity slots:
    k/v = entities @ {Wk,Wv}.T + b;  scores = q . k;  attn = softmax_e(scores)
    ctx = sum_e attn * v;            out = ctx @ Wo.T + bo

Algebraic reductions:
 1. The query is one shared vector, so the K projection folds into a tiny
    fused weight computed on host: scores[t, h] = X[t, :] @ M[:, h].
    (The bk term c_h is a per-(s,b,h) constant across entities, so it
    cancels exactly in the softmax and is dropped.)
 2. The entity-average commutes with the (linear) V projection:
       ctx[(s,b), d] = sum_c Wv[d, c] * Y[h(d), c, (s,b)],
       Y[h, c, (s,b)] = sum_e attn[s,b,h,e] * X[(s,e,b), c].

v3 dataflow (vs the [h, t'] scores layout of the baseline):
 - scores are computed TOKENS-ON-PARTITIONS: out[t', h] with the X^T
   chunk as the PE stationary operand and the small M chunk streaming
   (N=16 per matmul) -- 8x fewer PE column-cycles than streaming X^T.
 - softmax in [t', h] layout: exp on ACT (full 128 partitions); then
     wbu[t', (j,b',h)] = attn * blockdiag      (one DVE mul, masked+expanded)
     zs[1, (j,b',h)]   = ones^T @ wbu          (PE block-sum row, packed
                                                into the scores PSUM bank)
     zr = 1/zs (DVE); zb = partition_broadcast(zr) on the idle Pool engine
     wb = wbu * zb                             (all-SBUF fp16 DVE mul)
 - X^T comes from k_dma DMA'd chunks + (8-k_dma) PE-transposed chunks
   (k_dma=0: X ships once, all transposition on-chip).
 - ctx^T runs in w=32 sb-pieces after every other super-tile and the out
   projection in per-512-col halves, so epilogue PE bursts stay small;
   epilogue weights ride the Pool SWDGE DMA queue so the SP input stream
   never queues behind them.
Per-ST DVE/ACT cost collapses (no 16-partition softmax ops, no r4s
transpose matmul), cutting the modeled span 129us -> 103us.
"""

import numpy as np

import concourse.bass as bass
import concourse.tile as tile
from concourse import bacc, mybir, bass_utils

# Problem constants (from spec / setup_inputs)
D = 1024
H = 16
HD = D // H
N_SENTS = 32
N_ENTS = 32
SE = N_SENTS * N_ENTS
B = 64
N_CORES = 8
BC = B // N_CORES            # batch columns per core
TOK = N_SENTS * N_ENTS * BC  # tokens per core = 8192
ST_TOK = 512                 # tokens per super-tile (16 (s,b) x 32 e)
N_ST = TOK // ST_TOK         # 16 super-tiles
SB = N_SENTS * BC            # (s, b) rows per core = 256

F32 = mybir.dt.float32
F16 = mybir.dt.float16

# Single-blob input layout (column offsets into XB [128, NB_COLS], fp16).
XT_OFF = 0                       # [128, 8*TOK]
XN_OFF = XT_OFF + 8 * TOK        # [128, (TOK//128)*D]
WVT_OFF = XN_OFF + (TOK // 128) * D   # [128, 8*D]
WOT_OFF = WVT_OFF + 8 * D        # [128, 8*D]
MW_OFF = WOT_OFF + 8 * D         # [128, 8*H]
BM64_OFF = MW_OFF + 8 * H        # [128, 64]
BV_OFF = BM64_OFF + 64           # [128, 16] fp32 bytes as 16 fp16
BO_OFF = BV_OFF + 16             # [1, D]
MASKT_OFF = BO_OFF + D           # [128, 2*4*N_ST] fp32 bytes as fp16 pairs
ID_OFF = MASKT_OFF + 8 * N_ST    # [128, 128] identity (PE transpose rhs)
NB_COLS = ID_OFF + 128

# pipeline offsets (stage at which each phase of st issues)
ZS_LAG = 1    # block-sum matmul + reciprocal + partition broadcast
Y_LAG = 2     # normalize mul + Y matmuls + yt copies
N_STAGES = N_ST + Y_LAG
# ctx pieces attach after y(st): st -> piece args (piece, sb0, w, half, coff)
# w=32 pieces after every other ST spread the epilogue PE bursts evenly
PIECES_AT = {2 * k + 1: [(k, 32 * k, 32, k // 4, (32 * k) % 128)]
             for k in range(8)}
# out projection split per 512-col half: (half, nh) after pieces of y(st)
OUT_AT = {7: [(0, 0)], 9: [(0, 1)], 15: [(1, 0), (1, 1)]}

_NC_CACHE = {}


def _build(use_mask=True, repeat=1, k_dma=0, ready_first=True,
           xts_eng="v", yt_eng="s"):
    key = ("nc", use_mask, repeat, k_dma, ready_first, xts_eng, yt_eng)
    if key in _NC_CACHE:
        return _NC_CACHE[key]
    nc = bacc.Bacc("TRN2", target_bir_lowering=False, debug=False)

    XB = nc.dram_tensor("XB", [128, NB_COLS], F16, kind="ExternalInput").ap()
    XT = XB[:, XT_OFF:XT_OFF + 8 * TOK]
    XN = XB[:, XN_OFF:XN_OFF + (TOK // 128) * D]
    WVT = XB[:, WVT_OFF:WVT_OFF + 8 * D]
    WOT = XB[:, WOT_OFF:WOT_OFF + 8 * D]
    MW = XB[:, MW_OFF:MW_OFF + 8 * H]
    BM64 = XB[:, BM64_OFF:BM64_OFF + 64]
    BV16 = XB[:, BV_OFF:BV_OFF + 16]
    BO = XB[:1, BO_OFF:BO_OFF + D]
    MASKT = XB[:, MASKT_OFF:MASKT_OFF + 8 * N_ST]
    ID128 = XB[:, ID_OFF:ID_OFF + 128]
    OUT = nc.dram_tensor("OUT", [SB, D], F16, kind="ExternalOutput").ap()

    ntr = 8 - k_dma              # chunks transposed on-chip per ST

    with tile.TileContext(nc) as tc:
        with (
            tc.tile_pool(name="wpool", bufs=1) as wpool,
            tc.tile_pool(name="xtp", bufs=4) as xtp,
            tc.tile_pool(name="xnp", bufs=7) as xnp,
            tc.tile_pool(name="xtsp", bufs=2) as xtsp,
            tc.tile_pool(name="attnpool", bufs=3) as apool,
            tc.tile_pool(name="wbpool", bufs=3) as wbpool,
            tc.tile_pool(name="ctxpool", bufs=1) as cpool,
            tc.tile_pool(name="psS", bufs=2, space="PSUM") as psS,
            tc.tile_pool(name="psY", bufs=2, space="PSUM") as psY,
            tc.tile_pool(name="psCtx", bufs=1, space="PSUM") as psCtx,
            tc.tile_pool(name="psT", bufs=2, space="PSUM") as psT,
            tc.tile_pool(name="psF", bufs=1, space="PSUM") as psF,
        ):
            # ---- constants / weights. The big epilogue weights wvt/wot are
            # DMA'd mid-loop so early super-tile loads aren't queued behind
            # them. ----
            wvt = wpool.tile([128, 8 * D], F16)
            wot = wpool.tile([128, 8 * D], F16)
            mw = wpool.tile([128, 8 * H], F16)
            nc.sync.dma_start(mw[:], MW)
            bv16 = wpool.tile([128, 16], F16)
            bv = bv16[:].bitcast(F32)
            bo = wpool.tile([1, D], F16)
            bm64 = wpool.tile([128, 64], F16)
            ones = wpool.tile([1, 128], F16)
            nc.vector.memset(ones[:], 1.0)
            onesc = wpool.tile([128, 1], F16)
            nc.vector.memset(onesc[:], 1.0)
            id128 = wpool.tile([128, 128], F16)
            nc.sync.dma_start(id128[:], ID128)
            if use_mask:
                maskt16 = wpool.tile([128, 8 * N_ST], F16)
                maskt = maskt16[:].bitcast(F32)

            # Y accumulator: [c-in-chunk, cs(8) * (H * SB) + h * SB + sb]
            yt = cpool.tile([128, 8 * H * SB], F16)
            # ctx^T per half: [d-in-chunk, m0(8) * 128 sb-half]
            ctxT = [cpool.tile([128, 8 * 128], F16, tag=f"ctxT{i}",
                               name=f"ctxT{i}")
                    for i in range(2)]

            ps_t = [None] * N_ST
            wbu_t = [None] * N_ST
            zb_t = [None] * N_ST
            wb_t = [None] * N_ST
            xn_t = [None] * N_ST
            xt_t = [None] * N_ST

            def load_xt(st):
                if k_dma == 0:
                    return
                xt = xtp.tile([128, k_dma * ST_TOK], F16, tag="xt")
                nc.sync.dma_start(
                    xt[:], XT[:, st * 8 * ST_TOK:
                              st * 8 * ST_TOK + k_dma * ST_TOK])
                xt_t[st] = xt

            def load_xn(st):
                xn = xnp.tile([128, 4 * D], F16, tag="xn")
                nc.sync.dma_start(
                    xn[:], XN[:, st * 4 * D:(st + 1) * 4 * D])
                xn_t[st] = xn

            def stage_scores(st):
                load_xt(st)
                load_xn(st)
                xt = xt_t[st]
                xn = xn_t[st]
                if st == 0:
                    # small consts + epilogue weights ride the Pool SWDGE
                    # queue so the SP input stream (xt/xn) never queues
                    # behind them
                    nc.gpsimd.dma_start(bm64[:], BM64)
                    if use_mask:
                        nc.gpsimd.dma_start(maskt16[:], MASKT)
                    nc.gpsimd.dma_start(bv16[:], BV16)
                    nc.gpsimd.dma_start(bo[:], BO)
                if 1 <= st <= 2:   # wvt needed from the first ctx quarter
                    q = (st - 1) * 4
                    nc.gpsimd.dma_start(wvt[:, q * D:(q + 4) * D],
                                        WVT[:, q * D:(q + 4) * D])
                if 3 <= st <= 6:   # wot needed from the first out half
                    q = (st - 3) * 2
                    nc.gpsimd.dma_start(wot[:, q * D:(q + 2) * D],
                                        WOT[:, q * D:(q + 2) * D])

                # ---- on-chip X^T for c-chunks k_dma..7: PE-transpose the
                # xn tile through PSUM staging banks (2 chunks per pass) ----
                xts = None
                for g in range(ntr // 2):
                    if xts is None:
                        xts = xtsp.tile([128, ntr * ST_TOK], F16, tag="xts")
                    pst = psT.tile([128, 2 * ST_TOK], F16, tag="pst")
                    for kk in range(2):
                        cs = k_dma + g * 2 + kk
                        for tc_ in range(4):
                            nc.tensor.transpose(
                                pst[:, kk * ST_TOK + tc_ * 128:
                                    kk * ST_TOK + (tc_ + 1) * 128],
                                xn[:, tc_ * D + cs * 128:
                                   tc_ * D + (cs + 1) * 128],
                                id128[:],
                            )
                    dst = xts[:, g * 2 * ST_TOK:(g * 2 + 2) * ST_TOK]
                    e = xts_eng[g % len(xts_eng)]
                    if e == "v":
                        nc.vector.tensor_copy(dst, pst[:])
                    elif e == "d":
                        nc.scalar.dma_start(dst, pst[:])
                    else:
                        nc.scalar.copy(dst, pst[:])

                # ---- scores[t', h] per 128-token group: X^T chunk is the
                # stationary operand, M chunk streams (N=16).  The packed
                # tile also reserves cols 64:320 for the zs block-sum row
                # (PSUM tiles are bank-granular; packing avoids a bank). ----
                psk = psS.tile([128, 320], F32, tag="ps_s")
                ps_t[st] = psk
                ps_s = psk[:, 0:64]
                for g in range(4):
                    for c in range(8):
                        lhsT = (xt[:, c * ST_TOK + g * 128:
                                   c * ST_TOK + (g + 1) * 128]
                                if c < k_dma else
                                xts[:, (c - k_dma) * ST_TOK + g * 128:
                                    (c - k_dma) * ST_TOK + (g + 1) * 128])
                        nc.tensor.matmul(
                            ps_s[:, g * 16:(g + 1) * 16],
                            lhsT,
                            mw[:, c * H:(c + 1) * H],
                            start=(c == 0), stop=(c == 7),
                        )

                # ---- exp on full 128 partitions (ACT) ----
                attn_e = apool.tile([128, 64], F16, tag="attn")
                if use_mask:
                    # per-group exp with the token's mask value as the
                    # per-partition bias
                    for g in range(4):
                        nc.scalar.activation(
                            attn_e[:, g * 16:(g + 1) * 16],
                            ps_s[:, g * 16:(g + 1) * 16],
                            mybir.ActivationFunctionType.Exp,
                            bias=maskt[:, st * 4 + g: st * 4 + g + 1])
                else:
                    nc.scalar.activation(attn_e[:], ps_s[:],
                                         mybir.ActivationFunctionType.Exp)
                # ---- unnormalized masked+expanded weights:
                # wbu[t', (j,h,b')] = attn[t', (j,h)] * blockdiag[t', (h,b')]
                wbu = wbpool.tile([128, 256], F16, tag="wbu")
                nc.vector.tensor_mul(
                    wbu[:].rearrange("p (j b h) -> p j b h", j=4, b=4),
                    attn_e[:].rearrange("p (j h) -> p j h", j=4)
                    [:, :, None, :].broadcast_to((128, 4, 4, H)),
                    bm64[:].rearrange("p (b h) -> p b h", b=4)[:, None, :, :]
                    .broadcast_to((128, 4, 4, H)),
                )
                wbu_t[st] = wbu

            def stage_zs(st):
                # ---- block sums zs[1, (j,h,b')] = ones^T @ wbu (PE, into
                # the packed scores tile), reciprocal (DVE), and partition
                # broadcast to all 128 rows (Pool) ----
                zs1 = ps_t[st][0:1, 64:320]
                nc.tensor.matmul(zs1, onesc[:],
                                 wbu_t[st][:], start=True, stop=True)
                zr = apool.tile([1, 256], F16, tag="zr")
                with nc.allow_low_precision(reason="1/z fits fp16"):
                    nc.vector.reciprocal(zr[:], zs1)
                zb = apool.tile([128, 256], F16, tag="zb")
                nc.gpsimd.partition_broadcast(zb[:], zr[:])
                zb_t[st] = zb
                ps_t[st] = None

            def stage_wb(st):
                # ---- normalize: wb = wbu * zb (all-SBUF fp16 mul) ----
                wb = wbpool.tile([128, 256], F16, tag="wb")
                nc.vector.tensor_mul(wb[:], wbu_t[st][:], zb_t[st][:])
                wb_t[st] = wb
                wbu_t[st] = None
                zb_t[st] = None

            def stage_y(st):
                xn = xn_t[st]
                wb = wb_t[st]
                # ---- Y tiles: one PSUM bank holds a (cs, cs+1) pair ----
                ytv = yt[:].rearrange("p (ch h sb) -> p ch h sb", ch=8, h=H)
                for cp in range(4):
                    ps_y = psY.tile([128, 512], F32, tag="ps_y")
                    for kk in range(2):
                        cs = 2 * cp + kk
                        for j in range(4):
                            nc.tensor.matmul(
                                ps_y[:, kk * 256 + j * 64:
                                     kk * 256 + (j + 1) * 64],
                                xn[:, j * D + cs * 128:
                                   j * D + (cs + 1) * 128],
                                wb[:, j * 64:(j + 1) * 64],
                                start=True, stop=True,
                            )
                    # copy into yt: psum col kk*256 + j*64 + b'*16 + h
                    #  -> yt col (2cp+kk)*(H*SB) + h*SB + st*16 + j*4 + b'
                    src = ps_y[:].rearrange("p (k j bq h) -> p k h (j bq)",
                                            k=2, j=4, bq=4)
                    dst = ytv[:, 2 * cp: 2 * cp + 2, :,
                              st * 16: st * 16 + 16]
                    e = yt_eng[cp % len(yt_eng)]
                    if e == "v":
                        nc.vector.tensor_copy(dst, src)
                    else:
                        nc.scalar.copy(dst, src)
                xn_t[st] = None
                wb_t[st] = None

            def ctx_piece(piece, sb0, w, half, coff):
                # ---- ctx^T: [128 d (2 heads col-packed), w sb] ----
                ctxT_bf = ctxT[half]
                # all matmuls of a 4-m0 bank first, then its adds: a region's
                # reader would otherwise false-WAR the next region's matmuls
                for mg in range(2):
                    ps_ctx = psCtx.tile([128, 4 * w], F32, tag="ps_ctx")
                    for ml in range(4):
                        m0 = mg * 4 + ml
                        reg = ps_ctx[:, ml * w:(ml + 1) * w]
                        for hh in range(2):
                            h = 2 * m0 + hh
                            for c in range(8):
                                nc.tensor.matmul(
                                    reg[hh * 64:(hh + 1) * 64, :],
                                    wvt[:, c * D + h * HD: c * D + h * HD + HD],
                                    yt[:, c * (H * SB) + h * SB + sb0:
                                       c * (H * SB) + h * SB + sb0 + w],
                                    start=(c == 0), stop=(c == 7),
                                    tile_position=(0, hh * 64),
                                )
                    for ml in range(4):
                        m0 = mg * 4 + ml
                        reg = ps_ctx[:, ml * w:(ml + 1) * w]
                        dst = ctxT_bf[:, m0 * 128 + coff: m0 * 128 + coff + w]
                        if ml % 2 == 0:
                            nc.vector.tensor_add(
                                dst, reg,
                                bv[:, m0:m0 + 1].broadcast_to((128, w)))
                        else:
                            nc.scalar.activation(
                                dst, reg,
                                mybir.ActivationFunctionType.Identity,
                                bias=bv[:, m0:m0 + 1])

            fins = [None, None]

            def out_part(half, nh):
                # ---- out projection: OUT[sb, f] = ctx^T.T @ WoT + bo,
                # one 512-col half per call so the PE burst is split ----
                sb0 = half * 128
                ctxT_bf = ctxT[half]
                if fins[half] is None:
                    fins[half] = cpool.tile([128, D], F16, tag=f"fin{half}",
                                            name=f"fin{half}")
                fin = fins[half]
                ps_f = psF.tile([128, 512], F32, tag="ps_f")
                for c in range(8):
                    nc.tensor.matmul(
                        ps_f[:],
                        ctxT_bf[:, c * 128:(c + 1) * 128],
                        wot[:, c * D + nh * 512: c * D + (nh + 1) * 512],
                        start=(c == 0), stop=False,
                    )
                nc.tensor.matmul(
                    ps_f[:], ones[:, :128],
                    bo[:, nh * 512:(nh + 1) * 512],
                    start=False, stop=True,
                )
                nc.scalar.copy(fin[:, nh * 512:(nh + 1) * 512], ps_f[:])
                # OUT goes out on the ACT-engine DGE queue so it never
                # blocks the SP input stream.
                nc.scalar.dma_start(
                    OUT[sb0:sb0 + 128, nh * 512:(nh + 1) * 512],
                    fin[:, nh * 512:(nh + 1) * 512])

            def ready_work(stage):
                st_y = stage - Y_LAG
                if st_y >= 0 and st_y < N_ST:
                    stage_wb(st_y)
                    stage_y(st_y)
                    for args in PIECES_AT.get(st_y, []):
                        ctx_piece(*args)
                    for (hf, nh) in OUT_AT.get(st_y, []):
                        out_part(hf, nh)

            for rep in range(repeat):
                for stage in range(N_STAGES):
                    if ready_first:
                        ready_work(stage)
                        if stage < N_ST:
                            stage_scores(stage)
                        # zs of st-1 last: its exp/wbu deps had this whole
                        # stage's PE work to complete
                        if stage - ZS_LAG >= 0 and stage - ZS_LAG < N_ST:
                            stage_zs(stage - ZS_LAG)
                    else:
                        if stage < N_ST:
                            stage_scores(stage)
                        if stage - ZS_LAG >= 0 and stage - ZS_LAG < N_ST:
                            stage_zs(stage - ZS_LAG)
                        ready_work(stage)

    nc.compile()
    _NC_CACHE[key] = nc
    return nc


def _prep_host(entities, padding_mask, n_sents, query, in_proj_w, in_proj_b,
               out_proj_w, out_proj_b):
    """Host-side prep: shard + layout/dtype packing + weight fusion."""
    assert int(n_sents) == N_SENTS
    f16 = np.float16
    f32 = np.float32

    Wq = in_proj_w[:D]
    Wk = in_proj_w[D:2 * D]
    Wv = in_proj_w[2 * D:]
    bq = in_proj_b[:D]
    bv = in_proj_b[2 * D:]
    scale = np.float64(1.0) / np.sqrt(np.float64(HD))

    q_vec = ((query.astype(np.float64) @ Wq.T.astype(np.float64)
              + bq.astype(np.float64)) * scale)
    # M[c, h] = sum_hd q_vec[h*HD+hd] * Wk[h*HD+hd, c]
    # (the bk fold c_h is a constant across entities per (s,b,h): it
    # cancels in the softmax and is omitted)
    M = np.stack(
        [q_vec[h * HD:(h + 1) * HD] @ Wk.astype(np.float64)[h * HD:(h + 1) * HD, :]
         for h in range(H)], axis=1)  # [D, H]

    def pack_kxn(w_t):  # [1024, N] -> [128, 8*N] chunk-major
        n = w_t.shape[1]
        return np.ascontiguousarray(
            w_t.reshape(8, 128, n).transpose(1, 0, 2).reshape(128, 8 * n))

    WVT = pack_kxn(Wv.T.astype(f32)).astype(f16)
    WOT = pack_kxn(out_proj_w.T.astype(f32)).astype(f16)
    MW = pack_kxn(M.astype(f32)).astype(f16)
    BVp = np.ascontiguousarray(bv.astype(f32).reshape(8, 128).T)  # [128, 8]
    BOp = out_proj_b.astype(f32).reshape(1, D).astype(f16)

    # BM64[32*b1 + e, b2*16 + h] = [b1 == b2]
    BM64p = np.zeros((128, 64), dtype=f16)
    for b1 in range(4):
        BM64p[b1 * 32:(b1 + 1) * 32, b1 * 16:(b1 + 1) * 16] = 1.0
    ent16 = entities.astype(f16)  # [SE, B, D]
    maskf = padding_mask.astype(f32) * f32(-30000.0)

    in_maps = []
    for core in range(N_CORES):
        bsl = slice(core * BC, (core + 1) * BC)
        # token order (s, b, e): t = (s*BC + b)*NE + e
        xflat = np.ascontiguousarray(
            ent16[:, bsl, :].reshape(N_SENTS, N_ENTS, BC, D)
            .transpose(0, 2, 1, 3)).reshape(TOK, D)
        # X natural, super-tile-major: [p, st * 4096 + j * D + c]
        xn = np.ascontiguousarray(
            xflat.reshape(N_ST, 4, 128, D).transpose(2, 0, 1, 3)
            .reshape(128, N_ST * 4 * D))
        # X^T, super-tile-major: [p=c-in-chunk, st * 4096 + c_chunk * 512 + t]
        xt = xflat.T.reshape(8, 128, N_ST, ST_TOK)
        xt = np.ascontiguousarray(
            xt.transpose(1, 2, 0, 3).reshape(128, 8 * TOK))
        # mask in [t'(128), (st, g)] layout as f32 (exp bias operand)
        maskg = np.ascontiguousarray(
            maskf[:, bsl].reshape(N_SENTS, N_ENTS, BC).transpose(0, 2, 1)
            .reshape(TOK)).reshape(N_ST, 4, 128).transpose(2, 0, 1)
        maskg = np.ascontiguousarray(maskg.reshape(128, 4 * N_ST), dtype=f32)
        xb = np.zeros((128, NB_COLS), dtype=f16)
        xb[:, XT_OFF:XT_OFF + 8 * TOK] = xt
        xb[:, XN_OFF:XN_OFF + (TOK // 128) * D] = xn
        xb[:, WVT_OFF:WVT_OFF + 8 * D] = WVT
        xb[:, WOT_OFF:WOT_OFF + 8 * D] = WOT
        xb[:, MW_OFF:MW_OFF + 8 * H] = MW
        xb[:, BM64_OFF:BM64_OFF + 64] = BM64p
        xb[:, BV_OFF:BV_OFF + 16] = BVp.view(f16)
        xb[:1, BO_OFF:BO_OFF + D] = BOp
        xb[:, MASKT_OFF:MASKT_OFF + 8 * N_ST] = maskg.view(f16)
        xb[:, ID_OFF:ID_OFF + 128] = np.eye(128, dtype=f16)
        in_maps.append({"XB": xb})
    return in_maps


def kernel(entities, padding_mask, n_sents, query, in_proj_w, in_proj_b,
           out_proj_w, out_proj_b):
    # Accept jax/np arrays alike; host prep must run in numpy (and the
    # q/Wk fold in float64, which jax with x64 disabled would silently
    # downcast).
    entities = np.asarray(entities)
    padding_mask = np.asarray(padding_mask)
    query = np.asarray(query)
    in_proj_w = np.asarray(in_proj_w)
    in_proj_b = np.asarray(in_proj_b)
    out_proj_w = np.asarray(out_proj_w)
    out_proj_b = np.asarray(out_proj_b)
    n_sents = int(n_sents)
    in_maps = _prep_host(entities, padding_mask, n_sents, query, in_proj_w,
                         in_proj_b, out_proj_w, out_proj_b)
    nc = _build(use_mask=bool(np.any(padding_mask)))
    res = None
    last_err = None
    for attempt in range(3):
        try:
            res = bass_utils.run_bass_kernel_spmd(
                nc, in_maps=in_maps, core_ids=list(range(N_CORES)))
            break
        except Exception as e:  # rare transient device wedge; retry
            last_err = e
            import time as _time
            _time.sleep(3)
    if res is None:
        raise last_err
    out = np.empty((N_SENTS, B, D), dtype=np.float32)
    for core in range(N_CORES):
        o = res.results[core]["OUT"].astype(np.float32).reshape(
            N_SENTS, BC, D)
        out[:, core * BC:(core + 1) * BC, :] = o
    return out


# revision 4
# speedup vs baseline: 1.5863x; 1.5863x over previous
"""Trainium2 Bass kernel for nn_Aggregation (sparse block-diagonal attention).

Computation (see reference): a single learned query vector attends, per
(sentence, batch), over that sentence's 32 entity slots:
    k/v = entities @ {Wk,Wv}.T + b;  scores = q . k;  attn = softmax_e(scores)
    ctx = sum_e attn * v;            out = ctx @ Wo.T + bo

Algebraic reductions:
 1. The query is one shared vector, so the K projection folds into a tiny
    fused weight computed on host: scores[t, h] = X[t, :] @ M[:, h].
    (The bk term c_h is a per-(s,b,h) constant across entities, so it
    cancels exactly in the softmax and is dropped.)
 2. The entity-average commutes with the (linear) V projection:
       ctx[(s,b), d] = sum_c Wv[d, c] * Y[h(d), c, (s,b)],
       Y[h, c, (s,b)] = sum_e attn[s,b,h,e] * X[(s,e,b), c].

v3 dataflow (vs the [h, t'] scores layout of the baseline):
 - scores are computed TOKENS-ON-PARTITIONS: out[t', h] with the X^T
   chunk as the PE stationary operand and the small M chunk streaming
   (N=16 per matmul) -- 8x fewer PE column-cycles than streaming X^T.
 - softmax in [t', h] layout: exp on ACT (full 128 partitions); then
     wbu[t', (j,b',h)] = attn * blockdiag      (one DVE mul, masked+expanded)
     zs[1, (j,b',h)]   = ones^T @ wbu          (PE block-sum row, packed
                                                into the scores PSUM bank)
     zr = 1/zs (DVE); zb = partition_broadcast(zr) on the idle Pool engine
     wb = wbu * zb                             (all-SBUF fp16 DVE mul)
 - X^T comes from k_dma DMA'd chunks + (8-k_dma) PE-transposed chunks
   (k_dma=0: X ships once, all transposition on-chip).
 - ctx^T runs in w=32 sb-pieces after every other super-tile and the out
   projection in per-512-col halves, so epilogue PE bursts stay small;
   epilogue weights ride the Pool SWDGE DMA queue so the SP input stream
   never queues behind them.
Per-ST DVE/ACT cost collapses (no 16-partition softmax ops, no r4s
transpose matmul), cutting the modeled span 129us -> 103us.
"""

import numpy as np

import concourse.bass as bass
import concourse.tile as tile
from concourse import bacc, mybir, bass_utils

# Problem constants (from spec / setup_inputs)
D = 1024
H = 16
HD = D // H
N_SENTS = 32
N_ENTS = 32
SE = N_SENTS * N_ENTS
B = 64
N_CORES = 8
BC = B // N_CORES            # batch columns per core
TOK = N_SENTS * N_ENTS * BC  # tokens per core = 8192
ST_TOK = 512                 # tokens per super-tile (16 (s,b) x 32 e)
N_ST = TOK // ST_TOK         # 16 super-tiles
SB = N_SENTS * BC            # (s, b) rows per core = 256

F32 = mybir.dt.float32
F16 = mybir.dt.float16

# Single-blob input layout (column offsets into XB [128, NB_COLS], fp16).
XT_OFF = 0                       # [128, 8*TOK]
XN_OFF = XT_OFF + 8 * TOK        # [128, (TOK//128)*D]
WVT_OFF = XN_OFF + (TOK // 128) * D   # [128, 8*D]
WOT_OFF = WVT_OFF + 8 * D        # [128, 8*D]
MW_OFF = WOT_OFF + 8 * D         # [128, 8*H]
BM64_OFF = MW_OFF + 8 * H        # [128, 64]
BV_OFF = BM64_OFF + 64           # [128, 16] fp32 bytes as 16 fp16
BO_OFF = BV_OFF + 16             # [1, D]
MASKT_OFF = BO_OFF + D           # [128, 2*4*N_ST] fp32 bytes as fp16 pairs
ID_OFF = MASKT_OFF + 8 * N_ST    # [128, 128] identity (PE transpose rhs)
NB_COLS = ID_OFF + 128

# pipeline offsets (stage at which each phase of st issues)
ZS_LAG = 1    # block-sum matmul + reciprocal + partition broadcast
Y_LAG = 2     # normalize mul + Y matmuls + yt copies
N_STAGES = N_ST + Y_LAG
# ctx pieces attach after y(st): st -> piece args (piece, sb0, w, half, coff)
# w=32 pieces after every other ST spread the epilogue PE bursts evenly
PIECES_AT = {2 * k + 1: [(k, 32 * k, 32, k // 4, (32 * k) % 128)]
             for k in range(8)}
# out projection split per 512-col half: (half, nh) after pieces of y(st)
OUT_AT = {7: [(0, 0)], 9: [(0, 1)], 15: [(1, 0), (1, 1)]}

_NC_CACHE = {}


def _build(use_mask=True, repeat=1, k_dma=0, ready_first=True,
           xts_eng="v", yt_eng="s"):
    key = ("nc", use_mask, repeat, k_dma, ready_first, xts_eng, yt_eng)
    if key in _NC_CACHE:
        return _NC_CACHE[key]
    nc = bacc.Bacc("TRN2", target_bir_lowering=False, debug=False)

    XB = nc.dram_tensor("XB", [128, NB_COLS], F16, kind="ExternalInput").ap()
    XT = XB[:, XT_OFF:XT_OFF + 8 * TOK]
    XN = XB[:, XN_OFF:XN_OFF + (TOK // 128) * D]
    WVT = XB[:, WVT_OFF:WVT_OFF + 8 * D]
    WOT = XB[:, WOT_OFF:WOT_OFF + 8 * D]
    MW = XB[:, MW_OFF:MW_OFF + 8 * H]
    BM64 = XB[:, BM64_OFF:BM64_OFF + 64]
    BV16 = XB[:, BV_OFF:BV_OFF + 16]
    BO = XB[:1, BO_OFF:BO_OFF + D]
    MASKT = XB[:, MASKT_OFF:MASKT_OFF + 8 * N_ST]
    ID128 = XB[:, ID_OFF:ID_OFF + 128]
    OUT = nc.dram_tensor("OUT", [SB, D], F16, kind="ExternalOutput").ap()

    ntr = 8 - k_dma              # chunks transposed on-chip per ST

    with tile.TileContext(nc) as tc:
        with (
            tc.tile_pool(name="wpool", bufs=1) as wpool,
            tc.tile_pool(name="xtp", bufs=4) as xtp,
            tc.tile_pool(name="xnp", bufs=7) as xnp,
            tc.tile_pool(name="xtsp", bufs=2) as xtsp,
            tc.tile_pool(name="attnpool", bufs=3) as apool,
            tc.tile_pool(name="wbpool", bufs=3) as wbpool,
            tc.tile_pool(name="ctxpool", bufs=1) as cpool,
            tc.tile_pool(name="psS", bufs=2, space="PSUM") as psS,
            tc.tile_pool(name="psY", bufs=2, space="PSUM") as psY,
            tc.tile_pool(name="psCtx", bufs=1, space="PSUM") as psCtx,
            tc.tile_pool(name="psT", bufs=2, space="PSUM") as psT,
            tc.tile_pool(name="psF", bufs=1, space="PSUM") as psF,
        ):
            # ---- constants / weights. The big epilogue weights wvt/wot are
            # DMA'd mid-loop so early super-tile loads aren't queued behind
            # them. ----
            wvt = wpool.tile([128, 8 * D], F16)
            wot = wpool.tile([128, 8 * D], F16)
            mw = wpool.tile([128, 8 * H], F16)
            nc.sync.dma_start(mw[:], MW)
            bv16 = wpool.tile([128, 16], F16)
            bv = bv16[:].bitcast(F32)
            bo = wpool.tile([1, D], F16)
            bm64 = wpool.tile([128, 64], F16)
            ones = wpool.tile([1, 128], F16)
            nc.vector.memset(ones[:], 1.0)
            onesc = wpool.tile([128, 1], F16)
            nc.vector.memset(onesc[:], 1.0)
            # preload the ACT exp table during the initial DMA wait so the
            # first real exp doesn't pay the 1.3us table load
            actwarm = wpool.tile([1, 1], F16)
            nc.scalar.activation(actwarm[:], onesc[:1, :],
                                 mybir.ActivationFunctionType.Exp)
            id128 = wpool.tile([128, 128], F16)
            nc.sync.dma_start(id128[:], ID128)
            if use_mask:
                maskt16 = wpool.tile([128, 8 * N_ST], F16)
                maskt = maskt16[:].bitcast(F32)

            # Y accumulator: [c-in-chunk, cs(8) * (H * SB) + h * SB + sb]
            yt = cpool.tile([128, 8 * H * SB], F16)
            # ctx^T per half: [d-in-chunk, m0(8) * 128 sb-half]
            ctxT = [cpool.tile([128, 8 * 128], F16, tag=f"ctxT{i}",
                               name=f"ctxT{i}")
                    for i in range(2)]

            ps_t = [None] * N_ST
            wbu_t = [None] * N_ST
            zb_t = [None] * N_ST
            wb_t = [None] * N_ST
            xn_t = [None] * N_ST
            xt_t = [None] * N_ST

            def load_xt(st):
                if k_dma == 0:
                    return
                xt = xtp.tile([128, k_dma * ST_TOK], F16, tag="xt")
                nc.sync.dma_start(
                    xt[:], XT[:, st * 8 * ST_TOK:
                              st * 8 * ST_TOK + k_dma * ST_TOK])
                xt_t[st] = xt

            def load_xn(st):
                xn = xnp.tile([128, 4 * D], F16, tag="xn")
                nc.sync.dma_start(
                    xn[:], XN[:, st * 4 * D:(st + 1) * 4 * D])
                xn_t[st] = xn

            def stage_scores(st):
                load_xt(st)
                load_xn(st)
                xt = xt_t[st]
                xn = xn_t[st]
                if st == 0:
                    # small consts + epilogue weights ride the Pool SWDGE
                    # queue so the SP input stream (xt/xn) never queues
                    # behind them
                    nc.gpsimd.dma_start(bm64[:], BM64)
                    if use_mask:
                        nc.gpsimd.dma_start(maskt16[:], MASKT)
                    nc.gpsimd.dma_start(bv16[:], BV16)
                    nc.gpsimd.dma_start(bo[:], BO)
                if 1 <= st <= 2:   # wvt needed from the first ctx quarter
                    q = (st - 1) * 4
                    nc.gpsimd.dma_start(wvt[:, q * D:(q + 4) * D],
                                        WVT[:, q * D:(q + 4) * D])
                if 3 <= st <= 6:   # wot needed from the first out half
                    q = (st - 3) * 2
                    nc.gpsimd.dma_start(wot[:, q * D:(q + 2) * D],
                                        WOT[:, q * D:(q + 2) * D])

                # ---- on-chip X^T for c-chunks k_dma..7: PE-transpose the
                # xn tile through PSUM staging banks (2 chunks per pass) ----
                xts = None
                for g in range(ntr // 2):
                    if xts is None:
                        xts = xtsp.tile([128, ntr * ST_TOK], F16, tag="xts")
                    pst = psT.tile([128, 2 * ST_TOK], F16, tag="pst")
                    for kk in range(2):
                        cs = k_dma + g * 2 + kk
                        for tc_ in range(4):
                            nc.tensor.transpose(
                                pst[:, kk * ST_TOK + tc_ * 128:
                                    kk * ST_TOK + (tc_ + 1) * 128],
                                xn[:, tc_ * D + cs * 128:
                                   tc_ * D + (cs + 1) * 128],
                                id128[:],
                            )
                    dst = xts[:, g * 2 * ST_TOK:(g * 2 + 2) * ST_TOK]
                    e = xts_eng[g % len(xts_eng)]
                    if e == "v":
                        nc.vector.tensor_copy(dst, pst[:])
                    elif e == "d":
                        nc.scalar.dma_start(dst, pst[:])
                    else:
                        nc.scalar.copy(dst, pst[:])

                # ---- scores[t', h] per 128-token group: X^T chunk is the
                # stationary operand, M chunk streams (N=16).  The packed
                # tile also reserves cols 64:320 for the zs block-sum row
                # (PSUM tiles are bank-granular; packing avoids a bank). ----
                psk = psS.tile([128, 320], F32, tag="ps_s")
                ps_t[st] = psk
                ps_s = psk[:, 0:64]
                for g in range(4):
                    for c in range(8):
                        lhsT = (xt[:, c * ST_TOK + g * 128:
                                   c * ST_TOK + (g + 1) * 128]
                                if c < k_dma else
                                xts[:, (c - k_dma) * ST_TOK + g * 128:
                                    (c - k_dma) * ST_TOK + (g + 1) * 128])
                        nc.tensor.matmul(
                            ps_s[:, g * 16:(g + 1) * 16],
                            lhsT,
                            mw[:, c * H:(c + 1) * H],
                            start=(c == 0), stop=(c == 7),
                        )

                # ---- exp on full 128 partitions (ACT) ----
                attn_e = apool.tile([128, 64], F16, tag="attn")
                if use_mask:
                    # per-group exp with the token's mask value as the
                    # per-partition bias
                    for g in range(4):
                        nc.scalar.activation(
                            attn_e[:, g * 16:(g + 1) * 16],
                            ps_s[:, g * 16:(g + 1) * 16],
                            mybir.ActivationFunctionType.Exp,
                            bias=maskt[:, st * 4 + g: st * 4 + g + 1])
                else:
                    nc.scalar.activation(attn_e[:], ps_s[:],
                                         mybir.ActivationFunctionType.Exp)
                # ---- unnormalized masked+expanded weights:
                # wbu[t', (j,h,b')] = attn[t', (j,h)] * blockdiag[t', (h,b')]
                wbu = wbpool.tile([128, 256], F16, tag="wbu")
                nc.vector.tensor_mul(
                    wbu[:].rearrange("p (j b h) -> p j b h", j=4, b=4),
                    attn_e[:].rearrange("p (j h) -> p j h", j=4)
                    [:, :, None, :].broadcast_to((128, 4, 4, H)),
                    bm64[:].rearrange("p (b h) -> p b h", b=4)[:, None, :, :]
                    .broadcast_to((128, 4, 4, H)),
                )
                wbu_t[st] = wbu

            def stage_zs(st):
                # ---- block sums zs[1, (j,h,b')] = ones^T @ wbu (PE, into
                # the packed scores tile), reciprocal (DVE), and partition
                # broadcast to all 128 rows (Pool) ----
                zs1 = ps_t[st][0:1, 64:320]
                nc.tensor.matmul(zs1, onesc[:],
                                 wbu_t[st][:], start=True, stop=True)
                zr = apool.tile([1, 256], F16, tag="zr")
                with nc.allow_low_precision(reason="1/z fits fp16"):
                    nc.vector.reciprocal(zr[:], zs1)
                zb = apool.tile([128, 256], F16, tag="zb")
                nc.gpsimd.partition_broadcast(zb[:], zr[:])
                zb_t[st] = zb
                ps_t[st] = None

            def stage_wb(st):
                # ---- normalize: wb = wbu * zb (all-SBUF fp16 mul) ----
                wb = wbpool.tile([128, 256], F16, tag="wb")
                nc.vector.tensor_mul(wb[:], wbu_t[st][:], zb_t[st][:])
                wb_t[st] = wb
                wbu_t[st] = None
                zb_t[st] = None

            def stage_y(st):
                xn = xn_t[st]
                wb = wb_t[st]
                # ---- Y tiles: one PSUM bank holds a (cs, cs+1) pair ----
                ytv = yt[:].rearrange("p (ch h sb) -> p ch h sb", ch=8, h=H)
                for cp in range(4):
                    ps_y = psY.tile([128, 512], F32, tag="ps_y")
                    for kk in range(2):
                        cs = 2 * cp + kk
                        for j in range(4):
                            nc.tensor.matmul(
                                ps_y[:, kk * 256 + j * 64:
                                     kk * 256 + (j + 1) * 64],
                                xn[:, j * D + cs * 128:
                                   j * D + (cs + 1) * 128],
                                wb[:, j * 64:(j + 1) * 64],
                                start=True, stop=True,
                            )
                    # copy into yt: psum col kk*256 + j*64 + b'*16 + h
                    #  -> yt col (2cp+kk)*(H*SB) + h*SB + st*16 + j*4 + b'
                    src = ps_y[:].rearrange("p (k j bq h) -> p k h (j bq)",
                                            k=2, j=4, bq=4)
                    dst = ytv[:, 2 * cp: 2 * cp + 2, :,
                              st * 16: st * 16 + 16]
                    e = yt_eng[cp % len(yt_eng)]
                    if e == "v":
                        nc.vector.tensor_copy(dst, src)
                    else:
                        nc.scalar.copy(dst, src)
                xn_t[st] = None
                wb_t[st] = None

            def ctx_piece(piece, sb0, w, half, coff):
                # ---- ctx^T: [128 d (2 heads col-packed), w sb] ----
                ctxT_bf = ctxT[half]
                # all matmuls of a 4-m0 bank first, then its adds: a region's
                # reader would otherwise false-WAR the next region's matmuls
                for mg in range(2):
                    ps_ctx = psCtx.tile([128, 4 * w], F32, tag="ps_ctx")
                    for ml in range(4):
                        m0 = mg * 4 + ml
                        reg = ps_ctx[:, ml * w:(ml + 1) * w]
                        for hh in range(2):
                            h = 2 * m0 + hh
                            for c in range(8):
                                nc.tensor.matmul(
                                    reg[hh * 64:(hh + 1) * 64, :],
                                    wvt[:, c * D + h * HD: c * D + h * HD + HD],
                                    yt[:, c * (H * SB) + h * SB + sb0:
                                       c * (H * SB) + h * SB + sb0 + w],
                                    start=(c == 0), stop=(c == 7),
                                    tile_position=(0, hh * 64),
                                )
                    for ml in range(4):
                        m0 = mg * 4 + ml
                        reg = ps_ctx[:, ml * w:(ml + 1) * w]
                        dst = ctxT_bf[:, m0 * 128 + coff: m0 * 128 + coff + w]
                        if ml % 2 == 0:
                            nc.vector.tensor_add(
                                dst, reg,
                                bv[:, m0:m0 + 1].broadcast_to((128, w)))
                        else:
                            nc.scalar.activation(
                                dst, reg,
                                mybir.ActivationFunctionType.Identity,
                                bias=bv[:, m0:m0 + 1])

            fins = [None, None]

            def out_part(half, nh):
                # ---- out projection: OUT[sb, f] = ctx^T.T @ WoT + bo,
                # one 512-col half per call so the PE burst is split ----
                sb0 = half * 128
                ctxT_bf = ctxT[half]
                if fins[half] is None:
                    fins[half] = cpool.tile([128, D], F16, tag=f"fin{half}",
                                            name=f"fin{half}")
                fin = fins[half]
                ps_f = psF.tile([128, 512], F32, tag="ps_f")
                for c in range(8):
                    nc.tensor.matmul(
                        ps_f[:],
                        ctxT_bf[:, c * 128:(c + 1) * 128],
                        wot[:, c * D + nh * 512: c * D + (nh + 1) * 512],
                        start=(c == 0), stop=False,
                    )
                nc.tensor.matmul(
                    ps_f[:], ones[:, :128],
                    bo[:, nh * 512:(nh + 1) * 512],
                    start=False, stop=True,
                )
                nc.scalar.copy(fin[:, nh * 512:(nh + 1) * 512], ps_f[:])
                # OUT goes out on the ACT-engine DGE queue so it never
                # blocks the SP input stream.
                nc.scalar.dma_start(
                    OUT[sb0:sb0 + 128, nh * 512:(nh + 1) * 512],
                    fin[:, nh * 512:(nh + 1) * 512])

            def ready_work(stage):
                st_y = stage - Y_LAG
                if st_y >= 0 and st_y < N_ST:
                    stage_wb(st_y)
                    stage_y(st_y)
                    for args in PIECES_AT.get(st_y, []):
                        ctx_piece(*args)
                    for (hf, nh) in OUT_AT.get(st_y, []):
                        out_part(hf, nh)

            for rep in range(repeat):
                for stage in range(N_STAGES):
                    if ready_first:
                        ready_work(stage)
                        if stage < N_ST:
                            stage_scores(stage)
                        # zs of st-1 last: its exp/wbu deps had this whole
                        # stage's PE work to complete
                        if stage - ZS_LAG >= 0 and stage - ZS_LAG < N_ST:
                            stage_zs(stage - ZS_LAG)
                    else:
                        if stage < N_ST:
                            stage_scores(stage)
                        if stage - ZS_LAG >= 0 and stage - ZS_LAG < N_ST:
                            stage_zs(stage - ZS_LAG)
                        ready_work(stage)

    nc.compile()
    _NC_CACHE[key] = nc
    return nc


def _prep_host(entities, padding_mask, n_sents, query, in_proj_w, in_proj_b,
               out_proj_w, out_proj_b):
    """Host-side prep: shard + layout/dtype packing + weight fusion."""
    assert int(n_sents) == N_SENTS
    f16 = np.float16
    f32 = np.float32

    Wq = in_proj_w[:D]
    Wk = in_proj_w[D:2 * D]
    Wv = in_proj_w[2 * D:]
    bq = in_proj_b[:D]
    bv = in_proj_b[2 * D:]
    scale = np.float64(1.0) / np.sqrt(np.float64(HD))

    q_vec = ((query.astype(np.float64) @ Wq.T.astype(np.float64)
              + bq.astype(np.float64)) * scale)
    # M[c, h] = sum_hd q_vec[h*HD+hd] * Wk[h*HD+hd, c]
    # (the bk fold c_h is a constant across entities per (s,b,h): it
    # cancels in the softmax and is omitted)
    M = np.stack(
        [q_vec[h * HD:(h + 1) * HD] @ Wk.astype(np.float64)[h * HD:(h + 1) * HD, :]
         for h in range(H)], axis=1)  # [D, H]

    def pack_kxn(w_t):  # [1024, N] -> [128, 8*N] chunk-major
        n = w_t.shape[1]
        return np.ascontiguousarray(
            w_t.reshape(8, 128, n).transpose(1, 0, 2).reshape(128, 8 * n))

    WVT = pack_kxn(Wv.T.astype(f32)).astype(f16)
    WOT = pack_kxn(out_proj_w.T.astype(f32)).astype(f16)
    MW = pack_kxn(M.astype(f32)).astype(f16)
    BVp = np.ascontiguousarray(bv.astype(f32).reshape(8, 128).T)  # [128, 8]
    BOp = out_proj_b.astype(f32).reshape(1, D).astype(f16)

    # BM64[32*b1 + e, b2*16 + h] = [b1 == b2]
    BM64p = np.zeros((128, 64), dtype=f16)
    for b1 in range(4):
        BM64p[b1 * 32:(b1 + 1) * 32, b1 * 16:(b1 + 1) * 16] = 1.0
    ent16 = entities.astype(f16)  # [SE, B, D]
    maskf = padding_mask.astype(f32) * f32(-30000.0)

    in_maps = []
    for core in range(N_CORES):
        bsl = slice(core * BC, (core + 1) * BC)
        # token order (s, b, e): t = (s*BC + b)*NE + e
        xflat = np.ascontiguousarray(
            ent16[:, bsl, :].reshape(N_SENTS, N_ENTS, BC, D)
            .transpose(0, 2, 1, 3)).reshape(TOK, D)
        # X natural, super-tile-major: [p, st * 4096 + j * D + c]
        xn = np.ascontiguousarray(
            xflat.reshape(N_ST, 4, 128, D).transpose(2, 0, 1, 3)
            .reshape(128, N_ST * 4 * D))
        # X^T, super-tile-major: [p=c-in-chunk, st * 4096 + c_chunk * 512 + t]
        xt = xflat.T.reshape(8, 128, N_ST, ST_TOK)
        xt = np.ascontiguousarray(
            xt.transpose(1, 2, 0, 3).reshape(128, 8 * TOK))
        # mask in [t'(128), (st, g)] layout as f32 (exp bias operand)
        maskg = np.ascontiguousarray(
            maskf[:, bsl].reshape(N_SENTS, N_ENTS, BC).transpose(0, 2, 1)
            .reshape(TOK)).reshape(N_ST, 4, 128).transpose(2, 0, 1)
        maskg = np.ascontiguousarray(maskg.reshape(128, 4 * N_ST), dtype=f32)
        xb = np.zeros((128, NB_COLS), dtype=f16)
        xb[:, XT_OFF:XT_OFF + 8 * TOK] = xt
        xb[:, XN_OFF:XN_OFF + (TOK // 128) * D] = xn
        xb[:, WVT_OFF:WVT_OFF + 8 * D] = WVT
        xb[:, WOT_OFF:WOT_OFF + 8 * D] = WOT
        xb[:, MW_OFF:MW_OFF + 8 * H] = MW
        xb[:, BM64_OFF:BM64_OFF + 64] = BM64p
        xb[:, BV_OFF:BV_OFF + 16] = BVp.view(f16)
        xb[:1, BO_OFF:BO_OFF + D] = BOp
        xb[:, MASKT_OFF:MASKT_OFF + 8 * N_ST] = maskg.view(f16)
        xb[:, ID_OFF:ID_OFF + 128] = np.eye(128, dtype=f16)
        in_maps.append({"XB": xb})
    return in_maps


def kernel(entities, padding_mask, n_sents, query, in_proj_w, in_proj_b,
           out_proj_w, out_proj_b):
    # Accept jax/np arrays alike; host prep must run in numpy (and the
    # q/Wk fold in float64, which jax with x64 disabled would silently
    # downcast).
    entities = np.asarray(entities)
    padding_mask = np.asarray(padding_mask)
    query = np.asarray(query)
    in_proj_w = np.asarray(in_proj_w)
    in_proj_b = np.asarray(in_proj_b)
    out_proj_w = np.asarray(out_proj_w)
    out_proj_b = np.asarray(out_proj_b)
    n_sents = int(n_sents)
    in_maps = _prep_host(entities, padding_mask, n_sents, query, in_proj_w,
                         in_proj_b, out_proj_w, out_proj_b)
    nc = _build(use_mask=bool(np.any(padding_mask)))
    res = None
    last_err = None
    for attempt in range(3):
        try:
            res = bass_utils.run_bass_kernel_spmd(
                nc, in_maps=in_maps, core_ids=list(range(N_CORES)))
            break
        except Exception as e:  # rare transient device wedge; retry
            last_err = e
            import time as _time
            _time.sleep(3)
    if res is None:
        raise last_err
    out = np.empty((N_SENTS, B, D), dtype=np.float32)
    for core in range(N_CORES):
        o = res.results[core]["OUT"].astype(np.float32).reshape(
            N_SENTS, BC, D)
        out[:, core * BC:(core + 1) * BC, :] = o
    return out
